# revision 19
# baseline (speedup 1.0000x reference)
"""DeFeat distillation loss on 8 Trainium2 NeuronCores (Bass/Tile), v4.

Data-parallel over the batch dim (B=8 -> 1 batch element per core).

The loss only needs TWO scalars per pyramid level (masked / unmasked sums
of (t - Ws)^2), so the host permutes each level's pixels mask-first and
zero-pads both runs to a static 512-px block grid.  Every device-side
reduction covers a whole block (one accum column per block) and the host
maps blocks -> masked/background sums afterwards.  No per-pixel output.

Layout is channel-major ("flipped"): psum[oc, px].  Per 512-px block the
adaptation is ONE DoubleRow fp8 matmul per oc chunk (K=256 contraction in
a single pass; weights are the stationary operand so LDWEIGHTS hides
under the moving stream).  Two drain routes, alternating for balance:
  A : teacher subtract as a -16I matmul (PE), then one ACT
      Square+accum over the whole [128, 2W] psum tile.
  Bf: one fused custom-DVE op (registered below): accum += (t*16 - psum)
      squared, in a single pass over psum - no intermediate d tile.
psum tiles are 2 banks x bufs=4 so four blocks are in flight and the PE
streams continuously.  Blocks are fetched ~4 per DMA (~1 MB transfers)
alternating between the two DGE rings (sync HWDGE / gpsimd SWDGE).

fp8 scaling as v1: s, W, t' = 16*(t-b) all fp8e4m3 x16; psum holds 256*d;
the Square ops fold the rescale so every accum column is sum(d^2)
exactly.  Host sums columns, splits masked/bg per level, sqrt in f64.
"""

import os
import sys
from operator import add as _op_add

for _p in ("/opt/trn_rl_repo", os.path.expanduser("~/.axon_site/_ro/trn_rl_repo")):
    if os.path.isdir(_p) and _p not in sys.path:
        sys.path.insert(0, _p)

import numpy as np
import ml_dtypes

F8 = ml_dtypes.float8_e4m3
S_SCALE = 16.0

WEIGHT_GT = 0.004
WEIGHT_BG = 0.0002
STRIDES = (8, 16, 32, 64, 128)
SIZES = (128, 64, 32, 16, 8)
HWS = tuple(s * s for s in SIZES)          # (16384, 4096, 1024, 256, 64)
B, C, NBOX = 8, 256, 16
N_CORES = 8
N_LEVELS = 5

# Static block grid: per level the host lays the pixels out permuted
# [masked | unmasked] with NO padding (exactly HW pixels).  The one block
# per level that straddles the mask boundary gets a host-side fp32
# correction (cheap: <=512 px), so the grid is exact.
BLK_W = (512, 512, 512, 256, 64)           # block width per level
BLK_N = (32, 8, 2, 1, 1)                   # blocks per level (= HW / W)
BLOCKS = [(l, BLK_W[l]) for l in range(N_LEVELS) for _ in range(BLK_N[l])]
NBLK = len(BLOCKS)                         # 44
LVL_B0 = [sum(BLK_N[:l]) for l in range(N_LEVELS)]

# DMA chunks -> one dram tensor / one DMA each.  First chunk is a single
# block so compute starts as early as possible; the last chunks shrink to
# 2 blocks so the final compute isn't gated on one big transfer.
_CHUNK_IDS = [[0]]
_i = 1
while _i < NBLK:
    ids = list(range(_i, min(_i + 2, NBLK)))
    _CHUNK_IDS.append(ids)
    _i = ids[-1] + 1
CHUNKS = []
for cid, ids in enumerate(_CHUNK_IDS):
    offs = []
    tw = 0
    for k in ids:
        offs.append(tw)
        tw += BLOCKS[k][1]
    CHUNKS.append((cid, ids, offs, tw))

# Per-block route: "A" = PE negI subtract + ACT Square from psum;
# "Bf" = fused custom-DVE (t*16 - psum)^2 + accum, one pass.
# 21 A / 23 Bf balances measured per-block costs (ACT 1.39us vs DVE 1.27us).
_N_A = 21
ROUTES = ["A" if (i * _N_A) // NBLK != ((i + 1) * _N_A) // NBLK else "Bf"
          for i in range(NBLK)]


def _register_sqdiff_op():
    """Register the fused (in0 - in1*c0)^2 * c1 + row-accum DVE op using
    the documented custom-op extension point (concourse/dve_ops.py).
    Idempotent; sha computed at runtime so there is no drift."""
    import concourse.dve_ops as dops
    from concourse.dve_spec import (Spec, Src0, Src1, C0, C1, Zero, sq,
                                    lower, _has_src1)
    from concourse.dve_uop import DveOpSpec

    name = "SQDIFF_REDUCE_ANT"
    if name in dops._SUB_OPCODE_FOR_NAME:
        return next(op for op in dops.OPS if op.name == name)

    def _ref(in0, in1, s0, s1, imm2):
        b = (((in0.astype(np.float32) - in1 * s0) ** 2) * s1).astype(np.float32)
        return b, b.reshape(b.shape[0], -1).sum(axis=-1, keepdims=True)

    spec = Spec(body=sq(Src0 - Src1 * C0) * C1, accum=_op_add,
                accum_init=Zero, reference=_ref)
    row = dops._CUSTOM_DVE_ROW_BASE + len(dops.OPS)
    assert row < 0x20
    shas = {}
    for ver in ("v3", "v4"):
        t = DveOpSpec(name=name, opcode=row, uops=lower(spec, ver=ver),
                      rd1_en=_has_src1(spec))
        shas[ver] = t.sha(ver)
    op = dops.DveOp(name, spec, subdim=False, uops_sha=shas)
    dops._SUB_OPCODE_FOR_NAME[name] = row
    dops.OPS.append(op)
    dops.CUSTOM_DVE_SPECS[name] = spec
    return op


def _build_module():
    import concourse.mybir as mybir
    from concourse import bacc
    from concourse.tile import TileContext

    sqdiff = _register_sqdiff_op()

    dt = mybir.dt
    nc = bacc.Bacc("TRN2", target_bir_lowering=False, debug=False,
                   num_devices=N_CORES)

    ch_d = [nc.dram_tensor(f"ch{c}", [128, 4, tw], dt.float8e4,
                           kind="ExternalInput")
            for (c, _, _, tw) in CHUNKS]
    wt_d = nc.dram_tensor("wt", [128, 4 * N_LEVELS, 128], dt.float8e4,
                          kind="ExternalInput")
    ni_d = nc.dram_tensor("ni", [128, 128], dt.float8e4, kind="ExternalInput")
    out_q = nc.dram_tensor("out_q", [128, NBLK], dt.float32,
                           kind="ExternalOutput")

    SQUARE = mybir.ActivationFunctionType.Square
    DR = mybir.MatmulPerfMode.DoubleRow

    with TileContext(nc) as tc:
        with (
            tc.tile_pool(name="const", bufs=1) as const_pool,
            tc.tile_pool(name="feat", bufs=8) as feat_pool,
            tc.tile_pool(name="ps", bufs=4, space="PSUM") as psum_pool,
        ):
            wt = const_pool.tile([128, 4 * N_LEVELS, 128], dt.float8e4)
            ni = const_pool.tile([128, 128], dt.float8e4)
            qcat = const_pool.tile([128, NBLK], dt.float32)
            garb_a = const_pool.tile([128, 1024], dt.bfloat16)
            garb_v = const_pool.tile([128, 1024], dt.bfloat16)

            nc.sync.dma_start(out=wt[:], in_=wt_d[:])
            nc.sync.dma_start(out=ni[:], in_=ni_d[:])
            # warm the ACT Square table while the first chunks stream in
            nc.scalar.activation(garb_a[:, 0:1], ni[:, 0:1], SQUARE)

            for (cid, ids, offs, tw) in CHUNKS:
                ch = feat_pool.tile([128, 4, tw], dt.float8e4, tag="ch")
                nc.sync.dma_start(out=ch[:], in_=ch_d[cid][:])

                for k, boff in zip(ids, offs):
                    lvl, w = BLOCKS[k]
                    route = ROUTES[k]
                    ps = psum_pool.tile([128, 1024], dt.float32, tag="ps")
                    # adaptation matmuls (one DoubleRow K=256 per oc chunk)
                    for j in range(2):
                        nc.tensor.matmul(
                            ps[:, j * w:(j + 1) * w],
                            wt[:, 4 * lvl + 2 * j:4 * lvl + 2 * j + 2, :],
                            ch[:, 0:2, boff:boff + w],
                            start=True, stop=(route != "A"), perf_mode=DR)
                    if route == "A":
                        # psum -= 16 * t'' via -16I matmul
                        for j in range(2):
                            nc.tensor.matmul(
                                ps[:, j * w:(j + 1) * w],
                                ni[:],
                                ch[:, 2 + j, boff:boff + w],
                                start=False, stop=True)
                        nc.scalar.activation(
                            garb_a[:, 0:2 * w], ps[:, 0:2 * w], SQUARE,
                            scale=1.0 / 256.0, accum_out=qcat[:, k:k + 1])
                    else:
                        # accum += ((16t'' ) - psum/16)^2 / 256 = d^2
                        nc.vector._custom_dve(
                            sqdiff,
                            out=garb_v[:, 0:2 * w],
                            in0=ch[:, 2:4, boff:boff + w],
                            in1=ps[:, 0:2 * w],
                            s0=1.0 / 16.0, s1=1.0 / 256.0,
                            accum_out=qcat[:, k:k + 1])

                    # drain the early accum columns while late blocks run
                    if k == 27:
                        nc.sync.dma_start(out=out_q[:, 0:24], in_=qcat[:, 0:24])

            nc.sync.dma_start(out=out_q[:, 24:NBLK], in_=qcat[:, 24:NBLK])

    nc.compile()
    return nc


def _rasterize_masks(gt_bboxes):
    """Host-side mask rasterization, mirroring reference.gt_mask.

    Returns per-level [B, HW] bool masks."""
    out = []
    for lvl in range(N_LEVELS):
        h = w = SIZES[lvl]
        stride = np.float32(STRIDES[lvl])
        q = np.floor(gt_bboxes.astype(np.float32) / stride).astype(np.int32)
        lx = np.minimum(q[..., 0], w - 1)
        ly = np.minimum(q[..., 1], h - 1)
        rx = np.minimum(q[..., 2], w - 1)
        ry = np.minimum(q[..., 3], h - 1)
        lm = np.zeros((B, h * w), bool)
        for b in range(B):
            m = np.zeros((h, w), bool)
            for i in range(gt_bboxes.shape[1]):
                if lx[b, i] == rx[b, i] or ly[b, i] == ry[b, i]:
                    m[ly[b, i], lx[b, i]] = True
                else:
                    m[ly[b, i]:ry[b, i], lx[b, i]:rx[b, i]] = True
            lm[b] = m.reshape(-1)
        out.append(lm)
    return out


_NC_CACHE = None


def _get_nc():
    global _NC_CACHE
    if _NC_CACHE is None:
        _NC_CACHE = _build_module()
    return _NC_CACHE


def _run(in_maps, trace=False, trace_cores=None):
    from concourse.bass_utils import run_bass_kernel_spmd

    kwargs = {}
    if trace:
        kwargs.update(trace=True, trace_cores=trace_cores or [0])
    return run_bass_kernel_spmd(_get_nc(), in_maps, core_ids=list(range(N_CORES)),
                                **kwargs)


def _pack_wt(inputs):
    """wt[p, 4l+2j+i, m] = 16 * W_l[128j + m, 128i + p]."""
    wtp = np.zeros((128, 4 * N_LEVELS, 128), np.float32)
    for lvl in range(N_LEVELS):
        w = np.asarray(inputs[f"adapt_w{lvl}"], np.float32)
        for j in range(2):
            for i in range(2):
                wtp[:, 4 * lvl + 2 * j + i, :] = \
                    w[128 * j:128 * j + 128, 128 * i:128 * i + 128].T
    return (wtp * S_SCALE).astype(F8)


def _prep_in_maps(inputs, masks):
    """Per-core chunk arrays [128, 4, CW]: [s_ic0 | s_ic1 | t_oc0 | t_oc1],
    pixels permuted mask-first (no padding).  Returns (in_maps, binfo)
    where binfo[core][lvl] = (bb, q_corr): boundary block index within the
    level and the fp32 masked-prefix sum of that block."""
    wtp = _pack_wt(inputs)
    negi = (-S_SCALE * np.eye(128, dtype=np.float32)).astype(F8)
    binfo = [[None] * N_LEVELS for _ in range(N_CORES)]
    in_maps = []
    for b in range(N_CORES):
        m = {"wt": wtp, "ni": negi}
        lvl_st = []
        for lvl in range(N_LEVELS):
            hw, g = HWS[lvl], BLK_W[lvl]
            s = np.asarray(inputs[f"feat_s{lvl}"][b], np.float32).reshape(C, hw)
            bv = np.asarray(inputs[f"adapt_b{lvl}"], np.float32)
            wl = np.asarray(inputs[f"adapt_w{lvl}"], np.float32)
            t = np.asarray(inputs[f"feat_t{lvl}"][b], np.float32).reshape(C, hw)
            tp = t - bv[:, None]
            mask = masks[lvl][b]
            midx = np.flatnonzero(mask)
            nm = len(midx)
            perm = np.concatenate([midx, np.flatnonzero(~mask)])
            bb, cc = nm // g, nm % g
            if cc == 0:
                q_corr = 0.0
            else:
                pix = perm[bb * g:bb * g + cc]
                dcorr = tp[:, pix] - wl @ s[:, pix]
                q_corr = float((dcorr.astype(np.float64) ** 2).sum())
            binfo[b][lvl] = (bb, q_corr)
            st = np.empty((128, 4, hw), np.float32)
            sp = s[:, perm] * S_SCALE
            tpp = tp[:, perm] * S_SCALE
            st[:, 0] = sp[0:128]
            st[:, 1] = sp[128:256]
            st[:, 2] = tpp[0:128]
            st[:, 3] = tpp[128:256]
            lvl_st.append(st.astype(F8))
        blk_arr = []
        for k, (lvl, w) in enumerate(BLOCKS):
            i = k - LVL_B0[lvl]
            blk_arr.append(lvl_st[lvl][:, :, i * w:(i + 1) * w])
        for (cid, ids, offs, tw) in CHUNKS:
            m[f"ch{cid}"] = np.ascontiguousarray(
                np.concatenate([blk_arr[k] for k in ids], axis=2))
        in_maps.append(m)
    return in_maps, binfo


def kernel(_trace=False, _return_results=False, **inputs):
    gt_bboxes = np.asarray(inputs["gt_bboxes"], np.float32)
    masks = _rasterize_masks(gt_bboxes)
    in_maps, binfo = _prep_in_maps(inputs, masks)

    res = _run(in_maps, trace=_trace)

    loss = np.float64(0.0)
    for lvl in range(N_LEVELS):
        s_gt = np.float64(0.0)
        s_bg = np.float64(0.0)
        k0, nb = LVL_B0[lvl], BLK_N[lvl]
        for c in range(N_CORES):
            q = res.results[c]["out_q"].astype(np.float64)
            qb = q[:, k0:k0 + nb].sum(axis=0)
            bb, q_corr = binfo[c][lvl]
            # blocks < bb fully masked; block bb split via the host fp32
            # correction; blocks > bb fully unmasked.
            s_gt += qb[:bb].sum() + q_corr
            if bb < nb:
                s_bg += (qb[bb] - q_corr) + qb[bb + 1:].sum()
        loss += WEIGHT_GT * np.sqrt(s_gt + 1e-8) + \
            WEIGHT_BG * np.sqrt(s_bg + 1e-8)

    out = np.array(loss, dtype=np.float32)
    if _return_results:
        return out, res
    return out
